# revision 42
# baseline (speedup 1.0000x reference)
"""AdaMemAttention Trainium2 kernel (8 NeuronCores, SPMD) — v9, 261.6us.

Sharding: core c -> (batch b = c//2, head-group hg = c%2, heads hg*6..+6).
Tokens host-permuted per core to [own-half | other-half] so the SPMD
program is hg-invariant.

Design:
  - host pre-transposes x (xT, bf16) and memory-bank keys (memkT, f32r);
    DMA feeds matmuls directly (dram tensors typed f32r/bf16 so the
    HW verifier's "rounded to FP32r" producer rule is satisfied)
  - selection path (x0/wq1 -> q1, f32; memkT scores f32r) kept at high
    precision: top-k selection is the error-sensitive stage
  - qkv GEMM in bf16 (q1 has its own exact f32 path); attention in f32r
  - softmax exp split: Act seven 2-chunk groups (0-13); DVE chunks
    14-16 via dual-offset Schraudolph bit-trick (2x tensor_scalar to
    i32-bitcast f32 scratch + add to f32r, ~1% max rel err); GPSIMD
    does broadcasts/selection/gather only (no PSUM access, no ALU)
  - per-(ti,h) normalize rcp -> partition_broadcast -> mult (f32r out)
  - f32r ReduceScatter of the peer token-half, emission deferred and
    consumers on the idle SP queue so Act/DVE never block on it
  - software-pipelined attention: score(i) / exp(i-1) / av(i-AVSKEW),
    slow-exp singles interleaved between Act pair-groups (GORDER=2)
"""
import sys
sys.path.insert(0, "/opt/trn_rl_repo")
import numpy as np

B, N, C, H, D = 4, 1568, 768, 12, 64
NB, NP = 2048, 1568
NM = NB + NP
KB, KP = 153, 358
HL = 6
SEL = KB + KP              # 511
NH = N // 2                # 784

# dual-offset Schraudolph exp, i32/f32 bit-trick (exp(0.125*s));
# +0.5 because the f32->i32 store truncates
EXP_A32 = 8388608 * 1.4426950408889634 * 0.125
EXP_B32_1 = 1054309842.8 + 0.5
EXP_B32_2 = 1058422183.7 + 0.5

_cache = {}


def _build():
    import concourse.bass as bass
    import concourse.bacc as bacc
    import concourse.mybir as mybir
    import concourse.tile as tile

    dt = mybir.dt
    Alu = mybir.AluOpType
    Act = mybir.ActivationFunctionType
    f32, f32r, i16, i32, u32, bf16 = (dt.float32, dt.float32r,
                                      dt.int16, dt.int32,
                                      dt.uint32, dt.bfloat16)

    nc = bacc.Bacc("TRN2", target_bir_lowering=False, debug=False,
                   num_devices=8)

    x0_d = nc.dram_tensor("x0", [1, C], f32, kind="ExternalInput")
    xT_d = nc.dram_tensor("xT", [6, 128, N], bf16, kind="ExternalInput")
    wqkvT_d = nc.dram_tensor("wqkvT", [C, 1152], bf16,
                             kind="ExternalInput")
    wq1_d = nc.dram_tensor("wq1", [C, 384], f32, kind="ExternalInput")
    wprojT_d = nc.dram_tensor("wprojT", [C, C], f32r, kind="ExternalInput")
    bproj_d = nc.dram_tensor("bproj", [1, C], f32r, kind="ExternalInput")
    memkv_d = nc.dram_tensor("memkv", [HL, NM, 2 * D], f32,
                             kind="ExternalInput")
    memkT_d = nc.dram_tensor("memkT", [3, 128, NM], f32r,
                             kind="ExternalInput")
    ident_d = nc.dram_tensor("ident", [128, 128], f32, kind="ExternalInput")
    iota_d = nc.dram_tensor("iota226", [16, 226], f32, kind="ExternalInput")
    msk_d = nc.dram_tensor("msk", [128, 2], f32, kind="ExternalInput")

    out_d = nc.dram_tensor("out", [NH, C], f32, kind="ExternalOutput")

    AQT = [(NH, 512), (NH + 512, 272), (0, 512), (512, 272)]
    MC = [128] * 12 + [32] + [128, 128, 128, 127]
    NCH = 13
    QBANK = 1.0 - (KB - 0.5) / (NB - 1)
    QPREV = 1.0 - (KP - 0.5) / (NP - 1)
    QT = [512, 512, 512, 32]
    TOK0 = [0, 512, 1024, 1536]
    # exp groups: six 2-chunk Act groups, then single-chunk groups
    # ('dve2' = two bit-trick terms, summed by PE in the AV accumulation)
    import os
    GORDER = os.environ.get("KGORDER", "2")
    AVSKEW = int(os.environ.get("KAVSKEW", "4"))
    _acts = [("act", (2 * i, 2 * i + 1)) for i in range(6)]
    _sing = [("act", (12, 13)), ("dve", (14,)), ("dve", (15,)),
             ("dve", (16,))]
    if GORDER == "0":
        GDEF = _acts + _sing
    elif GORDER == "1":          # singles first
        GDEF = _sing + _acts
    else:                        # interleaved
        GDEF = [_sing[0], _acts[0], _sing[1], _acts[1], _sing[2], _acts[2],
                _sing[3], _acts[3], _acts[4], _acts[5]]
    KCOPY = os.environ.get("KCOPY", "act")
    FIRST_CHUNK = GDEF[0][1][0]
    LAST_GRP = len(GDEF) - 1

    with tile.TileContext(nc) as tc, \
         tc.tile_pool(name="cst", bufs=1) as cst, \
         tc.tile_pool(name="dram", bufs=1, space="DRAM") as dram, \
         tc.tile_pool(name="bigB", bufs=1) as bigB:

        ident = cst.tile([128, 128], f32)
        nc.scalar.dma_start(ident[:], ident_d[:])
        iota226 = cst.tile([16, 226], f32)
        nc.scalar.dma_start(iota226[:], iota_d[:])
        msk = cst.tile([128, 2], f32)
        nc.scalar.dma_start(msk[:], msk_d[:])
        ones1f = cst.tile([1, 128], f32)
        nc.vector.memset(ones1f[:], 1.0)
        ones1 = cst.tile([1, 128], f32r)
        nc.vector.tensor_copy(ones1[:], ones1f[:])
        q1 = cst.tile([128, 3], f32)
        q1blk = cst.tile([128, 6], f32r)

        qT = bigB.tile([128, 3, N], f32r)
        kTc = bigB.tile([128, 3, N], f32r)
        kTs = bigB.tile([128, 3, 512], f32r)
        v_cur = bigB.tile([128, NCH, HL, 65], f32r)
        v_sel = bigB.tile([128, 4, HL, 65], f32r)

        a_send = dram.tile([2, 128, 3, NH], f32r)
        a_recv = dram.tile([128, 3, NH], f32r)

        with tc.tile_pool(name="scA", bufs=1) as scA:
            xTr = scA.tile([128, 6, N], bf16)
            wqr = scA.tile([128, 6, 1152], bf16)
            kvsel = scA.tile([128, HL, 4, 128], f32)
            x0f = scA.tile([128, 6], f32)
            wq1 = scA.tile([128, 6, 384], f32)

            # ===== phase A: loads (host-transposed layouts) =====
            nc.sync.dma_start(
                x0f[:],
                x0_d[0:1, :].rearrange("one (cc p) -> p (one cc)", p=128))
            nc.sync.dma_start(
                wq1[:], wq1_d[:].rearrange("(cc p) f -> p cc f", p=128))

            with tc.tile_pool(name="kTp", bufs=1) as kTp, \
                 tc.tile_pool(name="pS", bufs=1) as pS, \
                 tc.tile_pool(name="scS", bufs=1) as scS:
                kThalf = {}

                def load_kth(j):
                    for hf, (c0, w) in enumerate(((0, 2048), (2048, 1568))):
                        t = kTp.tile([128, 2048], f32r, tag="kth",
                                     name=f"kth{j}{hf}", bufs=4)
                        nc.sync.dma_start(t[:, 0:w],
                                          memkT_d[j, :, c0:c0 + w])
                        kThalf[(j, hf)] = t

                def load_xt(t):
                    n0, nn = TOK0[t], QT[t]
                    nc.scalar.dma_start(
                        xTr[:, :, n0:n0 + nn],
                        xT_d[:, :, n0:n0 + nn].rearrange("cc p n -> p cc n"))

                load_kth(0)
                nc.sync.dma_start(
                    wqr[:, :, 0:384],
                    wqkvT_d[:, 0:384].rearrange("(cc p) f -> p cc f", p=128))
                load_xt(0)
                load_kth(1)
                load_xt(1)
                nc.sync.dma_start(
                    wqr[:, :, 384:1152],
                    wqkvT_d[:, 384:1152].rearrange("(cc p) f -> p cc f",
                                                   p=128))
                load_kth(2)
                load_xt(2)
                load_xt(3)

                # q1 projection
                with tc.tile_pool(name="psA", bufs=2, space="PSUM") as psA:
                    for fq in range(3):
                        q1ps = psA.tile([128, 1], f32, space="PSUM",
                                        tag="q1ps", name="q1ps")
                        for cc in range(6):
                            nc.tensor.matmul(
                                q1ps[:], wq1[:, cc, 128 * fq:128 * (fq + 1)],
                                x0f[:, cc:cc + 1],
                                start=(cc == 0), stop=(cc == 5))
                        nc.vector.tensor_copy(q1[:, fq:fq + 1], q1ps[:])
                nc.vector.memset(q1blk[:].bitcast(f32), 0.0)
                for h in range(HL):
                    hb = 64 * (h % 2)
                    nc.vector.tensor_copy(
                        q1blk[hb:hb + 64, h:h + 1],
                        q1[hb:hb + 64, h // 2:h // 2 + 1])
                for c in range(NCH):
                    rows = 128 if c < 12 else 32
                    nc.vector.memset(
                        v_cur[0:rows, c, :, 64:65].bitcast(f32), 1.0)
                for c in range(4):
                    rows = 128 if c < 3 else 127
                    nc.vector.memset(
                        v_sel[0:rows, c, :, 64:65].bitcast(f32), 1.0)

                # ===== phase S + C interleaved =====
                sct = scS.tile([66, NM], f32)
                kbt = scS.tile([128, 12, 16], f32)
                s16 = scS.tile([16, HL, 226], f32)
                thr12 = scS.tile([1, 24], f32)
                thrB = scS.tile([16, 24], f32)
                selall = scS.tile([16, HL, 32], f32)
                idxs = scS.tile([128, HL, 32], i16)
                nfound = scS.tile([1, 16], u32)
                nc.vector.memset(kbt[:], -1.0e30)
                nc.vector.memset(selall[:], 0.0)

                with tc.tile_pool(name="psS", bufs=2, space="PSUM") as psS, \
                     tc.tile_pool(name="psC", bufs=2, space="PSUM") as psC:

                    def emit_score_j(j):
                        for hf, base, chunks in (
                                (0, 0, (512, 512, 512, 512)),
                                (1, 2048, (512, 512, 512, 32))):
                            kth = kThalf[(j, hf)]
                            off = 0
                            for w in chunks:
                                scps = psS.tile([2, 512], f32, space="PSUM",
                                                tag="scps", name="scps",
                                                bufs=4)
                                nc.tensor.matmul(
                                    scps[0:2, 0:w],
                                    q1blk[:, 2 * j:2 * j + 2],
                                    kth[:, off:off + w],
                                    start=True, stop=True)
                                nc.scalar.copy(
                                    sct[32 * j:32 * j + 2,
                                        base + off:base + off + w],
                                    scps[0:2, 0:w])
                                off += w

                    def emit_select_j(j):
                        for hh in range(2):
                            h = 2 * j + hh
                            p = 32 * j + hh
                            nc.sync.dma_start(
                                kbt[:, h, 0:16],
                                sct[p:p + 1, 0:NB].rearrange(
                                    "one (p2 f) -> one p2 f", p2=128))
                            nc.sync.dma_start(
                                kbt[:, 6 + h, 0:12],
                                sct[p:p + 1, NB:NB + 1536].rearrange(
                                    "one (p2 f) -> one p2 f", p2=128))
                            nc.sync.dma_start(
                                kbt[0:32, 6 + h, 12:13],
                                sct[p:p + 1, NB + 1536:NM].rearrange(
                                    "one (p2 f) -> one p2 f", p2=32))
                            nc.sync.dma_start(
                                s16[:, h, 0:128],
                                sct[p:p + 1, 0:NB].rearrange(
                                    "one (p2 f) -> one p2 f", p2=16))
                            nc.sync.dma_start(
                                s16[:, h, 128:226],
                                sct[p:p + 1, NB:NM].rearrange(
                                    "one (p2 f) -> one p2 f", p2=16))
                            nc.gpsimd.kth_largest(
                                thr12[0:1, 2 * h:2 * h + 2],
                                kbt[:, h, 0:16], 16, KB, quantile=QBANK)
                            nc.gpsimd.kth_largest(
                                thr12[0:1, 12 + 2 * h:14 + 2 * h],
                                kbt[:, 6 + h, 0:13], 13, KP, quantile=QPREV)
                            nc.gpsimd.partition_broadcast(
                                thrB[:, 2 * h:2 * h + 2],
                                thr12[0:1, 2 * h:2 * h + 2])
                            nc.gpsimd.partition_broadcast(
                                thrB[:, 12 + 2 * h:14 + 2 * h],
                                thr12[0:1, 12 + 2 * h:14 + 2 * h])
                            tsel = pS.tile([16, 226], f32, tag="tsel",
                                           name="tsel", bufs=2)
                            nc.vector.scalar_tensor_tensor(
                                tsel[:, 0:128], s16[:, h, 0:128],
                                thrB[:, 2 * h:2 * h + 1],
                                iota226[:, 0:128],
                                op0=Alu.is_ge, op1=Alu.mult)
                            nc.vector.scalar_tensor_tensor(
                                tsel[:, 128:226], s16[:, h, 128:226],
                                thrB[:, 12 + 2 * h:13 + 2 * h],
                                iota226[:, 128:226],
                                op0=Alu.is_ge, op1=Alu.mult)
                            nc.vector.tensor_scalar_add(tsel[:], tsel[:],
                                                        -1.0)
                            nc.gpsimd.sparse_gather(
                                selall[:, h, :], tsel[:],
                                num_found=nfound[0:1, h:h + 1])
                        # clamp + gather this pair now
                        nc.vector.tensor_scalar(
                            selall[:, 2 * j:2 * j + 2, :],
                            selall[:, 2 * j:2 * j + 2, :],
                            0.0, float(NM - 1), Alu.max, Alu.min)
                        seli = pS.tile([16, 2, 32], i16, tag="seli",
                                       name="seli", bufs=2)
                        nc.vector.tensor_copy(seli[:],
                                              selall[:, 2 * j:2 * j + 2, :])
                        nc.sync.dma_start(
                            idxs[0:16, 2 * j:2 * j + 2, :], seli[:])
                        nc.sync.dma_start(
                            idxs[16:32, 2 * j:2 * j + 2, :],
                            idxs[0:16, 2 * j:2 * j + 2, :])
                        nc.sync.dma_start(
                            idxs[32:64, 2 * j:2 * j + 2, :],
                            idxs[0:32, 2 * j:2 * j + 2, :])
                        nc.sync.dma_start(
                            idxs[64:128, 2 * j:2 * j + 2, :],
                            idxs[0:64, 2 * j:2 * j + 2, :])
                        # KIDX marker
                        for hh in range(2):
                            h = 2 * j + hh
                            nc.gpsimd.dma_gather(
                                kvsel[:, h, :, :], memkv_d[h], idxs[:, h, :],
                                num_idxs=512, num_idxs_reg=512, elem_size=128)

                    def emit_gsel_j(j):
                        # selected-k transposes + v_sel extraction
                        for hh in range(2):
                            h = 2 * j + hh
                            for c in range(4):
                                kps = psC.tile([128, 512], f32, space="PSUM",
                                               tag="gqk", name="kps", bufs=2)
                                nc.tensor.transpose(
                                    kps[0:64, 0:128], kvsel[:, h, c, 0:64],
                                    ident[:])
                                nc.vector.tensor_copy(
                                    kTs[64 * hh:64 * hh + 64, j,
                                        128 * c:128 * (c + 1)],
                                    kps[0:64, 0:128])
                            nc.vector.tensor_copy(
                                v_sel[:, :, h, 0:64],
                                kvsel[:, h, :, 64:128])

                    def emit_qk_t(t):
                        n0, nn = TOK0[t], QT[t]
                        for fc in range(6):
                            dst = qT if fc < 3 else kTc
                            pair = fc % 3
                            g = psC.tile([128, 512], f32, space="PSUM",
                                         tag="gqk", name="gqk", bufs=2)
                            for cc in range(6):
                                nc.tensor.matmul(
                                    g[:, 0:nn],
                                    wqr[:, cc, 128 * fc:128 * (fc + 1)],
                                    xTr[:, cc, n0:n0 + nn],
                                    start=(cc == 0), stop=(cc == 5))
                            nc.vector.tensor_copy(dst[:, pair, n0:n0 + nn],
                                                  g[:, 0:nn])

                    def emit_v_t(t):
                        for c in range(4 * t, min(4 * t + 4, NCH)):
                            rows = 128 if c < 12 else 32
                            gv = psC.tile([128, 384], f32, space="PSUM",
                                          tag="gv", name="gv", bufs=2)
                            for cc in range(6):
                                nc.tensor.matmul(
                                    gv[0:rows, :],
                                    xTr[:, cc, 128 * c:128 * c + rows],
                                    wqr[:, cc, 768:1152],
                                    start=(cc == 0), stop=(cc == 5))
                            nc.vector.tensor_copy(
                                v_cur[0:rows, c, :, 0:64],
                                gv[0:rows, :].rearrange("p (h e) -> p h e",
                                                        h=HL))

                    emit_score_j(0)
                    emit_qk_t(0)
                    emit_select_j(0)
                    emit_v_t(0)
                    emit_score_j(1)
                    emit_qk_t(1)
                    emit_select_j(1)
                    emit_v_t(1)
                    emit_score_j(2)
                    emit_qk_t(2)
                    emit_select_j(2)
                    emit_v_t(2)
                    emit_qk_t(3)
                    emit_v_t(3)
                    emit_gsel_j(0)
                    emit_gsel_j(1)
                    emit_gsel_j(2)
        # scA closed

        # ===== phase H: software-pipelined single-pass attention =====
        # first two head-streams: selection-independent Act groups first,
        # so attention starts before the top-k gather chain completes
        GDEF_EARLY = sorted(GDEF, key=lambda g: max(g[1]) >= NCH + 1)
        NEARLY = int(os.environ.get("KNEARLY", "2"))
        groups = []
        for ti, (n0, nn) in enumerate(AQT):
            for h in range(HL):
                gdef = GDEF_EARLY if (ti == 0 and h < NEARLY) else GDEF
                for gidx, (eng, chunks) in enumerate(gdef):
                    groups.append((ti, h, gidx, eng, chunks, n0, nn))
        NG = len(groups)

        with tc.tile_pool(name="scH", bufs=1) as scH:
            aT = scH.tile([128, 3, NH], f32r)
            sendr = scH.tile([128, 3, NH], f32r)
            wpr = scH.tile([128, 6, C], f32r)
            bpr = scH.tile([1, C], f32r)
            aTfb = scH.tile([128, 3, NH], f32r)
            sc_tiles = {}
            pbt_tiles = {}
            ot_tiles = {}

            def klhs(cidx, hh, pr, mm):
                if cidx < NCH:
                    return kTc[hh:hh + 64, pr, 128 * cidx:128 * cidx + mm]
                sc0 = cidx - NCH
                return kTs[hh:hh + 64, pr, 128 * sc0:128 * sc0 + mm]

            def emit_score(i):
                ti, h, gidx, eng, chunks, n0, nn = groups[i]
                hh, pr = 64 * (h % 2), h // 2
                if gidx == 0:
                    ot_tiles[(ti, h)] = psH.tile([65, 512], f32, space="PSUM",
                                                 tag="ot", name="ot", bufs=2)
                if eng == "act":
                    sc_ = psH.tile([128, 1024], f32, space="PSUM",
                                   tag="scA", name="scA", bufs=2)
                else:
                    sc_ = psH.tile([128, 512], f32, space="PSUM",
                                   tag="scB", name="scB", bufs=2)
                sc_tiles[i] = sc_
                for gi, cidx in enumerate(chunks):
                    mm = MC[cidx]
                    nc.tensor.matmul(
                        sc_[0:mm, 512 * gi:512 * gi + nn],
                        klhs(cidx, hh, pr, mm),
                        qT[hh:hh + 64, pr, n0:n0 + nn],
                        start=True, stop=True)

            def emit_exp(i):
                ti, h, gidx, eng, chunks, n0, nn = groups[i]
                sc_ = sc_tiles.pop(i)
                if eng == "act":
                    pbt = pH.tile([128, 1024], f32r, tag="pbtA", name="pbtA",
                                  bufs=2 + AVSKEW)
                    nc.scalar.activation(
                        pbt[:].rearrange("p (g f) -> p g f",
                                         g=2)[:, :, 0:nn],
                        sc_[:].rearrange("p (g f) -> p g f",
                                         g=2)[:, :, 0:nn],
                        Act.Exp, scale=0.125)
                elif eng == "act1":
                    pbt = pH.tile([128, 512], f32r, tag="pbtB", name="pbtB",
                                  bufs=1 + AVSKEW)
                    nc.scalar.activation(pbt[:, 0:nn], sc_[:, 0:nn],
                                         Act.Exp, scale=0.125)
                else:
                    pbt = pH.tile([128, 512], f32r, tag="pbtT", name="pbtT",
                                  bufs=2 + AVSKEW)
                    s1 = pH.tile([128, 512], f32, tag="scrD1",
                                 name="scrD1", bufs=2)
                    s2 = pH.tile([128, 512], f32, tag="scrD2",
                                 name="scrD2", bufs=2)
                    nc.vector.tensor_scalar(
                        s1[:].bitcast(i32)[:, 0:nn], sc_[:, 0:nn],
                        EXP_A32, EXP_B32_1, Alu.mult, Alu.add)
                    nc.vector.tensor_scalar(
                        s2[:].bitcast(i32)[:, 0:nn], sc_[:, 0:nn],
                        EXP_A32, EXP_B32_2, Alu.mult, Alu.add)
                    nc.vector.tensor_tensor(
                        pbt[:, 0:nn], s1[:, 0:nn], s2[:, 0:nn],
                        op=Alu.add)
                pbt_tiles[i] = pbt

            def emit_av(i):
                ti, h, gidx, eng, chunks, n0, nn = groups[i]
                pbt = pbt_tiles.pop(i)
                ot = ot_tiles[(ti, h)]
                hh, pr = 64 * (h % 2), h // 2
                for gi, cidx in enumerate(chunks):
                    mm = MC[cidx]
                    vl = (v_cur[0:mm, cidx, h, :] if cidx < NCH
                          else v_sel[0:mm, cidx - NCH, h, :])
                    nc.tensor.matmul(
                        ot[:, 0:nn], vl, pbt[0:mm, 512 * gi:512 * gi + nn],
                        start=(gidx == 0 and gi == 0),
                        stop=(gidx == LAST_GRP and gi == len(chunks) - 1))
                if gidx != LAST_GRP:
                    return
                # normalize this (ti, h): rcp -> broadcast -> mult (v3 style)
                rcp = pH.tile([1, 512], f32r, tag="rcp", name="rcp", bufs=2)
                with nc.allow_low_precision(reason="f32r recip for PE"):
                    nc.vector.reciprocal(rcp[0:1, 0:nn], ot[64:65, 0:nn])
                rsb = pH.tile([64, 512], f32r, tag="rsb", name="rsb", bufs=2)
                nc.gpsimd.partition_broadcast(rsb[0:64, 0:nn],
                                              rcp[0:1, 0:nn])
                if ti >= 2:
                    nc.vector.tensor_tensor(aT[hh:hh + 64, pr, n0:n0 + nn],
                                            ot[0:64, 0:nn], rsb[0:64, 0:nn],
                                            op=Alu.mult)
                else:
                    nc.vector.tensor_tensor(
                        sendr[hh:hh + 64, pr, n0 - NH:n0 - NH + nn],
                        ot[0:64, 0:nn], rsb[0:64, 0:nn], op=Alu.mult)
                ot_tiles.pop((ti, h))
                if ti == 1 and h == HL - 1:
                    for s in range(2):
                        aTm = pH.tile([128, 3, NH], f32r, tag="aTm",
                                      name="aTm", bufs=1)
                        nc.vector.tensor_scalar_mul(
                            aTm[:].bitcast(f32), sendr[:].bitcast(f32),
                            msk[:, s:s + 1])
                        nc.sync.dma_start(a_send[s], aTm[:])
                if ti == 2 and h == 2:
                    nc.gpsimd.collective_compute(
                        "ReduceScatter", Alu.add,
                        ins=[a_send[:].opt()],
                        outs=[a_recv[:].opt()],
                        replica_groups=[[0, 1], [2, 3], [4, 5], [6, 7]],
                    )
                    nc.sync.dma_start(
                        wpr[:],
                        wprojT_d[:].rearrange("(cc p) f -> p cc f", p=128))
                    nc.sync.dma_start(bpr[:], bproj_d[:])
                    nc.sync.dma_start(aTfb[:], a_recv[:])

            with tc.tile_pool(name="pH", bufs=1) as pH, \
                 tc.tile_pool(name="psH", bufs=1, space="PSUM") as psH:
                for i in range(NG + AVSKEW):
                    if i < NG:
                        emit_score(i)
                    if 1 <= i <= NG:
                        emit_exp(i - 1)
                    if i >= AVSKEW:
                        emit_av(i - AVSKEW)

            # ===== phase P: projection =====
            with tc.tile_pool(name="pP", bufs=1) as pP, \
                 tc.tile_pool(name="psP", bufs=2, space="PSUM") as psP:
                for t in range(7):
                    rows = 128 if t < 6 else 16
                    yps = psP.tile([128, C], f32, space="PSUM", tag="yps",
                                   name="yps", bufs=2)
                    for c0, c1 in ((0, 512), (512, 768)):
                        nc.tensor.matmul(yps[0:rows, c0:c1],
                                         ones1[0:1, 0:rows],
                                         bpr[:, c0:c1], start=True,
                                         stop=False)
                        for cc in range(3):
                            nc.tensor.matmul(
                                yps[0:rows, c0:c1],
                                aT[:, cc, 128 * t:128 * t + rows],
                                wpr[:, cc, c0:c1],
                                start=False, stop=False)
                        for cc in range(3):
                            nc.tensor.matmul(
                                yps[0:rows, c0:c1],
                                aTfb[:, cc, 128 * t:128 * t + rows],
                                wpr[:, 3 + cc, c0:c1],
                                start=False, stop=(cc == 2))
                    ysb = pP.tile([128, C], f32, tag="ysb", name="ysb",
                                  bufs=2)
                    nc.scalar.copy(ysb[0:rows, :], yps[0:rows, :])
                    nc.sync.dma_start(out_d[128 * t:128 * t + rows, :],
                                      ysb[0:rows, :])

    nc.finalize()
    return nc


def _consts():
    ident = np.eye(128, dtype=np.float32)
    iota = np.zeros((16, 226), np.float32)
    for p in range(16):
        for f in range(128):
            iota[p, f] = p * 128 + f + 1
        for f in range(98):
            iota[p, 128 + f] = NB + p * 98 + f + 1
    return {"ident": ident, "iota226": iota}


def _get_nc():
    if "nc" not in _cache:
        _cache["nc"] = _build()
    return _cache["nc"]


def make_in_maps(x, bank_k, bank_v, prev_k, prev_v, w_qkv, w_proj, b_proj):
    import ml_dtypes
    bf16 = ml_dtypes.bfloat16
    x = np.asarray(x, np.float32)
    bank_k = np.asarray(bank_k, np.float32)
    bank_v = np.asarray(bank_v, np.float32)
    prev_k = np.asarray(prev_k, np.float32)
    prev_v = np.asarray(prev_v, np.float32)
    w_qkv = np.asarray(w_qkv, np.float32)
    w_proj = np.asarray(w_proj, np.float32)
    b_proj = np.asarray(b_proj, np.float32)
    consts = _consts()
    wprojT_full = np.ascontiguousarray(w_proj.T)
    in_maps = []
    for c in range(8):
        b, hg = c // 2, c % 2
        rows = np.concatenate([
            w_qkv[hg * 384:(hg + 1) * 384],
            w_qkv[C + hg * 384:C + (hg + 1) * 384],
            w_qkv[2 * C + hg * 384:2 * C + (hg + 1) * 384]], axis=0)
        own, oth = hg * NH, (1 - hg) * NH
        x_local = np.concatenate([x[b, own:own + NH], x[b, oth:oth + NH]],
                                 axis=0)
        xT = np.ascontiguousarray(x_local.T).reshape(6, 128, N)
        wp_local = np.concatenate([
            wprojT_full[hg * 384:(hg + 1) * 384],
            wprojT_full[(1 - hg) * 384:(2 - hg) * 384]], axis=0)
        memk = np.concatenate([bank_k[b, 6 * hg:6 * hg + 6],
                               prev_k[b, 6 * hg:6 * hg + 6]], axis=1)
        memv = np.concatenate([bank_v[b, 6 * hg:6 * hg + 6],
                               prev_v[b, 6 * hg:6 * hg + 6]], axis=1)
        memkv = np.concatenate([memk, memv], axis=2)   # [6, 3616, 128]
        memkT = np.empty((3, 128, NM), np.float32)
        for j in range(3):
            memkT[j, 0:64] = memk[2 * j].T
            memkT[j, 64:128] = memk[2 * j + 1].T
        mskv = np.zeros((128, 2), np.float32)
        mskv[:, 1 - hg] = 1.0
        wqT = np.ascontiguousarray(rows.T)
        m = {
            "x0": np.ascontiguousarray(x[b, 0:1, :]),
            "xT": xT.astype(bf16),
            "wqkvT": wqT.astype(bf16),
            "wq1": np.ascontiguousarray(wqT[:, 0:384]),
            "wprojT": np.ascontiguousarray(wp_local),
            "bproj": b_proj.reshape(1, C),
            "memkv": np.ascontiguousarray(memkv),
            "memkT": memkT,
            "msk": mskv,
        }
        m.update(consts)
        in_maps.append(m)
    return in_maps


def kernel(x, bank_k, bank_v, prev_k, prev_v, w_qkv, w_proj, b_proj,
           _trace=False):
    from concourse.bass_utils import run_bass_kernel_spmd
    nc = _get_nc()
    in_maps = make_in_maps(x, bank_k, bank_v, prev_k, prev_v,
                           w_qkv, w_proj, b_proj)
    res = run_bass_kernel_spmd(nc, in_maps, core_ids=list(range(8)),
                               trace=_trace)
    out = np.zeros((B, N, C), np.float32)
    for c in range(8):
        b, hg = c // 2, c % 2
        out[b, hg * NH:(hg + 1) * NH, :] = res.results[c]["out"]
    if _trace:
        return out, res
    return out
